# revision 12
# baseline (speedup 1.0000x reference)
"""Trainium2 Bass kernel for nn_ConvDatapath: quantized bit-sliced crossbar conv.

Pipeline (per core, data-parallel over Nx=6272 im2col rows, 784 rows/core):
  host: im2col (pure layout) -> xf [784, 580] per core (zero-padded K 576->580)
  device:
    1. per-row unsigned 8-bit quantization of x rows and w rows
       (min/max/sum reductions, q = rint((v-min)*inv) via the 2^23 magic-add
       trick fused into one ScalarE activation)
    2. PE-transpose of quantized (M+q) tiles into [K_block, rows] layout
    3. bit-slice into 4x 2-bit planes (int32 shift/and), convert to bf16
    4. 80 matmuls (5 K-blocks x 4 w-slices x 4 x-slices) [116]x[116,128]
       accumulating exact small-int products in PSUM f32
    5. ADC quantization 4*round(z/4) exactly via ScalarE activation
       Relu(z*(c/4) + c*M) with c = 4*WSF[ws]*ISF[is] (power of two), then
       DVE scalar_tensor_tensor (t - c*M) + acc accumulates the recombined
       integer Z exactly in f32 (|Z| < 2^24)
    6. dequant: Z*xs*ws + rank-2 offset correction via a tiny K=2 matmul
  host: gather per-core [128, 784] outputs -> [2,128,56,56]

All integer arithmetic is exact in f32; the only deviations from the jax
reference are sub-ulp rounding-tie differences in the quantizer ratio.
"""
import sys

sys.path.insert(0, "/opt/trn_rl_repo")

import numpy as np

# ---- problem constants (hardcoded per contract) ----
B, CIN, H, W_ = 2, 64, 56, 56
COUT, KH, KW = 128, 3, 3
K = CIN * KH * KW            # 576
NB, NPB = 5, 116             # chunker: 5 blocks of 116 (pad 4)
KPAD = NB * NPB              # 580
NCORES = 8
NX = B * H * W_              # 6272
R = NX // NCORES             # 784 rows per core
RT = 112                     # row tile -> 7 tiles per core
NJ = R // RT                 # 7
MAGIC = float(2 ** 23)
WSF = [64.0, 16.0, 4.0, 1.0]
ISF = [64.0, 16.0, 4.0, 1.0]
SH = [6, 4, 2, 0]            # slice shifts

_NC_CACHE = {}


def _build_program():
    import concourse.bass as bass
    import concourse.bacc as bacc
    import concourse.tile as tile
    from concourse import mybir
    from concourse.masks import make_identity

    f32 = mybir.dt.float32
    i32 = mybir.dt.int32
    bf16 = mybir.dt.bfloat16
    AF = mybir.ActivationFunctionType
    OP = mybir.AluOpType
    AX = mybir.AxisListType

    nc = bacc.Bacc("TRN2", target_bir_lowering=False, debug=False)

    d_xf = nc.dram_tensor("xf", (R, KPAD), f32, kind="ExternalInput")
    d_wf = nc.dram_tensor("wf", (COUT, KPAD), f32, kind="ExternalInput")
    d_out = nc.dram_tensor("out", (COUT, R), f32, kind="ExternalOutput")

    with tile.TileContext(nc) as tc:
        with (
            tc.tile_pool(name="const", bufs=1) as cpool,
            tc.tile_pool(name="work", bufs=2) as work,
            tc.tile_pool(name="stage", bufs=4) as stage,
            tc.tile_pool(name="psum", bufs=2, space="PSUM") as pps,
            tc.tile_pool(name="psz", bufs=3, space="PSUM") as psz,
        ):
            ident = cpool.tile([128, 128], f32)
            make_identity(nc, ident[:])

            # per-(ws,is) ADC bias constants c*M
            biasMC = cpool.tile([128, 16], f32)
            for wsi in range(4):
                for isi in range(4):
                    c = 4.0 * WSF[wsi] * ISF[isi]
                    nc.vector.memset(biasMC[:, wsi * 4 + isi : wsi * 4 + isi + 1], c * MAGIC)

            # ---------------- W prep ----------------
            w_sb = work.tile([COUT, KPAD], f32)
            nc.sync.dma_start(w_sb[:], d_wf.ap())
            w_min = cpool.tile([COUT, 1], f32)
            w_max = work.tile([COUT, 1], f32)
            w_sum = work.tile([COUT, 1], f32)
            nc.vector.tensor_reduce(w_min[:], w_sb[:], axis=AX.X, op=OP.min)
            nc.vector.tensor_reduce(w_max[:], w_sb[:], axis=AX.X, op=OP.max)
            nc.vector.tensor_reduce(w_sum[:], w_sb[:], axis=AX.X, op=OP.add)
            w_scale = cpool.tile([COUT, 1], f32)
            w_rng = work.tile([COUT, 1], f32)
            nc.vector.tensor_tensor(w_rng[:], w_max[:], w_min[:], op=OP.subtract)
            nc.vector.tensor_scalar(w_scale[:], w_rng[:], float(np.float32(1.0/255.0)), None, op0=OP.mult)
            w_inv = cpool.tile([COUT, 1], f32)
            nc.vector.reciprocal(w_inv[:], w_scale[:])
            Mtile = cpool.tile([128, 1], f32)
            nc.vector.memset(Mtile[:], MAGIC)
            w_negmin = work.tile([COUT, 1], f32)
            nc.vector.tensor_scalar(w_negmin[:], w_min[:], -1.0, None, op0=OP.mult)
            w_vr = work.tile([COUT, KPAD], f32)
            nc.scalar.activation(w_vr[:], w_sb[:], AF.Relu, bias=w_negmin[:], scale=1.0)

            qMw = work.tile([COUT, KPAD], f32)
            nc.scalar.activation(qMw[:], w_vr[:], AF.Relu, bias=Mtile[:], scale=w_inv[:])
            nc.vector.memset(qMw[:, K:KPAD], MAGIC)

            # wsl[b][ws]: [116, 128] bf16 stationary operands
            wslb = [[cpool.tile([NPB, COUT], bf16, tag=f"wsl{b}_{s}", name=f"wsl{b}_{s}") for s in range(4)]
                    for b in range(NB)]
            for b in range(NB):
                ps_t = pps.tile([NPB, COUT], f32, tag="ps_tr")
                nc.tensor.transpose(ps_t[:], qMw[:, b * NPB:(b + 1) * NPB], ident[:])
                wQT = work.tile([NPB, COUT], f32, tag="wQT")
                nc.scalar.copy(wQT[:], ps_t[:])
                wqi = wQT[:].bitcast(i32)
                for s in range(4):
                    wsl_i = work.tile([NPB, COUT], i32, tag="wsl_i")
                    if SH[s]:
                        nc.vector.tensor_scalar(wsl_i[:], wqi, SH[s], 3,
                                                op0=OP.logical_shift_right, op1=OP.bitwise_and)
                    else:
                        nc.vector.tensor_scalar(wsl_i[:], wqi, 3, None, op0=OP.bitwise_and)
                    nc.vector.tensor_copy(wslb[b][s][:], wsl_i[:])

            # correction row vectors: U1 = w_sum - 576*w_min ; U2 = w_min
            Upair = work.tile([COUT, 2], f32)
            nc.vector.scalar_tensor_tensor(Upair[:, 0:1], w_min[:], -576.0, w_sum[:],
                                           op0=OP.mult, op1=OP.add)
            nc.vector.tensor_copy(Upair[:, 1:2], w_min[:])
            ps_u = pps.tile([2, COUT], f32, tag="ps_tr")
            nc.tensor.transpose(ps_u[:], Upair[:], ident[:])
            UT = cpool.tile([2, COUT], f32)
            nc.scalar.copy(UT[:], ps_u[:])

            # ---------------- X prep ----------------
            QTx = [cpool.tile([NPB, R], f32, tag=f"QTx{b}", name=f"QTx{b}") for b in range(NB)]
            Vrow = cpool.tile([2, R], f32)   # rows: x_min, x_sum
            Vxs = cpool.tile([1, R], f32)    # x_scale row

            for j in range(NJ):
                x_sb = stage.tile([RT, KPAD], f32, tag="x_sb")
                nc.sync.dma_start(x_sb[:], d_xf.ap()[j * RT:(j + 1) * RT, :])
                xmin = stage.tile([RT, 1], f32, tag="xmin")
                xmax = stage.tile([RT, 1], f32, tag="xmax")
                xsum = stage.tile([RT, 1], f32, tag="xsum")
                nc.vector.tensor_reduce(xmin[:], x_sb[:], axis=AX.X, op=OP.min)
                nc.vector.tensor_reduce(xmax[:], x_sb[:], axis=AX.X, op=OP.max)
                nc.vector.tensor_reduce(xsum[:], x_sb[:], axis=AX.X, op=OP.add)
                xrng = stage.tile([RT, 1], f32, tag="xrng")
                nc.vector.tensor_tensor(xrng[:], xmax[:], xmin[:], op=OP.subtract)
                xscale = stage.tile([RT, 1], f32, tag="xscale")
                nc.vector.tensor_scalar(xscale[:], xrng[:], float(np.float32(1.0/255.0)), None, op0=OP.mult)
                xinv = stage.tile([RT, 1], f32, tag="xinv")
                nc.vector.reciprocal(xinv[:], xscale[:])
                xnegmin = stage.tile([RT, 1], f32, tag="xnegmin")
                nc.vector.tensor_scalar(xnegmin[:], xmin[:], -1.0, None, op0=OP.mult)
                x_vr = stage.tile([RT, KPAD], f32, tag="x_vr")
                nc.scalar.activation(x_vr[:], x_sb[:], AF.Relu, bias=xnegmin[:], scale=1.0)

                qMx = stage.tile([RT, KPAD], f32, tag="qMx")
                nc.scalar.activation(qMx[:], x_vr[:], AF.Relu, bias=Mtile[:RT], scale=xinv[:])
                nc.vector.memset(qMx[:, K:KPAD], MAGIC)

                # stats triple -> V rows via transpose
                Vtri = stage.tile([RT, 2], f32, tag="Vtri")
                nc.vector.tensor_copy(Vtri[:, 0:1], xmin[:])
                nc.vector.tensor_copy(Vtri[:, 1:2], xsum[:])
                ps_v = pps.tile([2, RT], f32, tag="ps_tr")
                nc.tensor.transpose(ps_v[:], Vtri[:], ident[:RT, :RT])
                nc.scalar.copy(Vrow[:, j * RT:(j + 1) * RT], ps_v[:])
                ps_x = pps.tile([1, RT], f32, tag="ps_tr")
                nc.tensor.transpose(ps_x[:], xscale[:], ident[:RT, :RT])
                nc.scalar.copy(Vxs[:, j * RT:(j + 1) * RT], ps_x[:])

                for b in range(NB):
                    ps_q = pps.tile([NPB, RT], f32, tag="ps_tr")
                    nc.tensor.transpose(ps_q[:], qMx[:, b * NPB:(b + 1) * NPB], ident[:RT, :RT])
                    nc.scalar.copy(QTx[b][:, j * RT:(j + 1) * RT], ps_q[:])

            # bit-slice planes, bf16
            xslb = [[cpool.tile([NPB, R], bf16, tag=f"xsl{b}_{s}", name=f"xsl{b}_{s}") for s in range(4)]
                    for b in range(NB)]
            for b in range(NB):
                xqi = QTx[b][:].bitcast(i32)
                for s in range(4):
                    xsl_i = work.tile([NPB, R], i32, tag="xsl_i")
                    if SH[s]:
                        nc.vector.tensor_scalar(xsl_i[:], xqi, SH[s], 3,
                                                op0=OP.logical_shift_right, op1=OP.bitwise_and)
                    else:
                        nc.vector.tensor_scalar(xsl_i[:], xqi, 3, None, op0=OP.bitwise_and)
                    if s % 2 == 0:
                        nc.vector.tensor_copy(xslb[b][s][:], xsl_i[:])
                    else:
                        nc.vector.tensor_copy(xslb[b][s][:], xsl_i[:])

            # ---------------- main loop ----------------
            out_t = cpool.tile([COUT, R], f32)
            HR = R // 2  # 392
            first = True
            for b in range(NB):
                for wsi in range(4):
                    for isi in range(4):
                        zps = psz.tile([128, 2, 512], f32, tag="zps")
                        nc.tensor.matmul(zps[:, 0, :HR], wslb[b][wsi][:],
                                         xslb[b][isi][:, 0:HR], start=True, stop=True)
                        nc.tensor.matmul(zps[:, 1, :HR], wslb[b][wsi][:],
                                         xslb[b][isi][:, HR:R], start=True, stop=True)
                        c = 4.0 * WSF[wsi] * ISF[isi]
                        tst = stage.tile([COUT, R], f32, tag="tst")
                        tst3 = tst[:].rearrange("p (a n) -> p a n", a=2)
                        nc.scalar.activation(tst3, zps[:, :, :HR], AF.Relu,
                                             bias=biasMC[:, wsi * 4 + isi: wsi * 4 + isi + 1],
                                             scale=c / 4.0)
                        if first:
                            nc.vector.tensor_scalar(out_t[:], tst[:], c * MAGIC, None,
                                                    op0=OP.subtract)
                            first = False
                        else:
                            eng = nc.vector
                            eng.scalar_tensor_tensor(out_t[:], tst[:], c * MAGIC, out_t[:],
                                                     op0=OP.subtract, op1=OP.add)

            # ---------------- dequant + corrections ----------------
            cps = psz.tile([128, 2, 512], f32, tag="zps")
            nc.tensor.matmul(cps[:, 0, :HR], UT[:], Vrow[0:2, 0:HR], start=True, stop=True)
            nc.tensor.matmul(cps[:, 1, :HR], UT[:], Vrow[0:2, HR:R], start=True, stop=True)

            # xs broadcast along partitions via ones-outer-product
            ones1 = cpool.tile([1, COUT], f32)
            nc.vector.memset(ones1[:], 1.0)
            xs_ps = psz.tile([128, 2, 512], f32, tag="zps")
            nc.tensor.matmul(xs_ps[:, 0, :HR], ones1[:], Vxs[:, 0:HR], start=True, stop=True)
            nc.tensor.matmul(xs_ps[:, 1, :HR], ones1[:], Vxs[:, HR:R], start=True, stop=True)

            outf = work.tile([COUT, R], f32)
            outf3 = outf[:].rearrange("p (a n) -> p a n", a=2)
            out_t3 = out_t[:].rearrange("p (a n) -> p a n", a=2)
            nc.vector.scalar_tensor_tensor(outf3, out_t3, w_scale[:], xs_ps[:, :, :HR],
                                           op0=OP.mult, op1=OP.mult)
            nc.vector.tensor_tensor(outf3, outf3, cps[:, :, :HR], op=OP.add)
            nc.sync.dma_start(d_out.ap(), outf[:])

    nc.compile()
    return nc


def _get_nc():
    if "nc" not in _NC_CACHE:
        _NC_CACHE["nc"] = _build_program()
    return _NC_CACHE["nc"]


def _im2col_host(x):
    # 3x3 SAME patches, column order [Cin, kh, kw]; rows (b, h, w)
    xp = np.pad(x, ((0, 0), (0, 0), (1, 1), (1, 1)))  # [B, C, 58, 58]
    s = xp.strides
    v = np.lib.stride_tricks.as_strided(
        xp,
        shape=(B, H, W_, CIN, KH, KW),
        strides=(s[0], s[2], s[3], s[1], s[2], s[3]),
    )
    return v.reshape(NX, K)


def kernel(x, w):
    from concourse.bass_utils import run_bass_kernel_spmd

    nc = _get_nc()
    x = np.ascontiguousarray(np.asarray(x, dtype=np.float32))
    w = np.asarray(w, dtype=np.float32)

    xf = np.zeros((NX, KPAD), np.float32)
    xf[:, :K] = _im2col_host(x)
    wf = np.zeros((COUT, KPAD), np.float32)
    wf[:, :K] = w.reshape(COUT, K)

    in_maps = [{"xf": np.ascontiguousarray(xf[c * R:(c + 1) * R]), "wf": wf}
               for c in range(NCORES)]
    import os
    trace = bool(os.environ.get("CONV_KERNEL_TRACE"))
    try:
        res = run_bass_kernel_spmd(nc, in_maps, core_ids=list(range(NCORES)), trace=trace)
    except Exception:
        if not trace:
            raise
        res = run_bass_kernel_spmd(nc, in_maps, core_ids=list(range(NCORES)), trace=False)
    _NC_CACHE["last_results"] = res
    z = np.concatenate([res.results[c]["out"].T for c in range(NCORES)], axis=0)
    return np.ascontiguousarray(
        z.reshape(B, H, W_, COUT).transpose(0, 3, 1, 2).astype(np.float32))
